# revision 1
# baseline (speedup 1.0000x reference)
"""JointNet (DGCNN geoNet + skeleton edge convs + joint MLP) on 8 trn2 cores.

Sharding: data-parallel over batch (B=4). Core c computes sample c % 4
end-to-end; cores 4-7 are redundant replicas (outputs of cores 0-3 are used).

Key kernel ideas:
- kNN: per 128-query tile, s[i,j] = h_i . h_j - xx_j/2 (row-constant terms
  dropped; ordering-equivalent to -distance). Matmul in float32r (full PE
  rate), with the -xx/2 term folded in as a rank-1 accumulating matmul.
  Top-20 = 3 rounds of DVE max8 / max_index / match_replace.
- Edge conv: leaky(max_k(feat @ w)) with feat = [neigh-center, center] and
  w = [wn | wc] decomposes (leaky monotone, max over k order-invariant) to
  leaky(max_k Y[idx_k] + Z) with Y = h @ wn.T (gathered via SWDGE dma_gather
  from HBM) and Z = h @ (wc-wn).T + b.
- Skeleton stage gathers via host-precomputed one-hot matmuls (J=24, tiny).
"""
import sys
if "/opt/trn_rl_repo" not in sys.path:
    sys.path.insert(0, "/opt/trn_rl_repo")

import numpy as np

import concourse.bacc as bacc
import concourse.tile_utils as tile_utils
tile_utils.max_sbuf_usage = 204 * 1024
import concourse.mybir as mybir
import concourse.tile as tile
from concourse.bass_utils import run_bass_kernel_spmd

f32 = mybir.dt.float32
f32r = mybir.dt.float32r
bf16 = mybir.dt.bfloat16
u16 = mybir.dt.uint16
i16 = mybir.dt.int16

N = 4096
NT = 32            # query tiles of 128
K = 20
J = 24
SLOPE = 0.2
LAYERS = [(3, 64), (64, 128), (128, 256)]   # (C_in, O) for g1..g3
NEG = -3.0e38

Copy = mybir.ActivationFunctionType.Copy
Square = mybir.ActivationFunctionType.Square
Amax = mybir.AluOpType.max
Aadd = mybir.AluOpType.add
Amult = mybir.AluOpType.mult


def _r(ap):
    return ap  # plain fp32 matmul (f32r needs producer-side rounding)


def _leaky(nc, out_ap, in_ap, tmp_ap):
    # out = max(0.2*x, x)
    nc.vector.tensor_scalar(tmp_ap, in_ap, SLOPE, None, Amult)
    nc.vector.tensor_tensor(out_ap, tmp_ap, in_ap, op=Amax)


def _emit(nc, tc, ins, outs):
    from contextlib import ExitStack
    ctx = ExitStack()
    sb1 = ctx.enter_context(tc.tile_pool(name="pers", bufs=1))
    sp = ctx.enter_context(tc.tile_pool(name="sp", bufs=2))
    wk = ctx.enter_context(tc.tile_pool(name="wk", bufs=2))
    gtp = ctx.enter_context(tc.tile_pool(name="gtp", bufs=2))
    skw = ctx.enter_context(tc.tile_pool(name="skw", bufs=6))
    skt = ctx.enter_context(tc.tile_pool(name="skt", bufs=4))
    lb = ctx.enter_context(tc.tile_pool(name="lb", bufs=1))
    pdp = ctx.enter_context(tc.tile_pool(name="pdp", bufs=3, space="PSUM"))
    pms = ctx.enter_context(tc.tile_pool(name="pms", bufs=2, space="PSUM"))
    pac = ctx.enter_context(tc.tile_pool(name="pac", bufs=1, space="PSUM"))
    psk = ctx.enter_context(tc.tile_pool(name="psk", bufs=2, space="PSUM"))
    dr = ctx.enter_context(tc.tile_pool(name="dr", bufs=1, space="DRAM"))

    def load(name, shape, tag=None):
        t = sb1.tile(shape, f32, tag=tag or name)
        nc.sync.dma_start(out=t[:], in_=ins[name].ap())
        return t

    eye = load("eye", [128, 128])
    selx = load("selx", [97, 4, 128])
    vt = load("vt", [3, N], tag="hcol2")
    vrows = load("vrows", [128, NT, 3])
    wtt = load("wtt", [128, NT, J])
    rw = load("rw", [J, 1])
    gw = []
    for li, (C, O) in enumerate(LAYERS, start=1):
        gw.append((load(f"g{li}_wnT", [C, O]), load(f"g{li}_zcT", [C, O]),
                   load(f"g{li}_b", [1, O])))
    skT = load("skT", [J, 4, J])
    sw_ = []
    for si, (ci, co) in enumerate([(512, 256), (256, 128), (128, 64)], start=1):
        sw_.append((load(f"s{si}_wnT", [128, ci // 128, co]),
                    load(f"s{si}_zcT", [128, ci // 128, co]),
                    load(f"s{si}_b", [1, co])))
    m1T = load("m1T", [128, 4, 512]); m1b = load("m1b", [1, 512])
    m2T = load("m2T", [128, 4, 256]); m2b = load("m2b", [1, 256])
    m3T = load("m3T", [128, 2, 3]); m3b = load("m3b", [1, 3])

    ones_r = sb1.tile([1, 128], f32, tag="ones_r")
    nc.vector.memset(ones_r[:], 1.0)
    ones_c = sb1.tile([128, 1], f32, tag="ones_c")
    nc.vector.memset(ones_c[:], 1.0)
    wttb = sb1.tile([128, NT, J], bf16, tag="wttb")
    nc.vector.tensor_copy(wttb[:], wtt[:])

    feat = sb1.tile([128, NT, 451], bf16, tag="feat")
    nc.vector.tensor_copy(feat[:, :, 0:3], vrows[:])

    hcol1 = sb1.tile([64, N], f32, tag="hcol1")
    hcol2 = sb1.tile([128, N], f32, tag="hcol2")
    hcols = [vt, hcol1, hcol2, None]
    foffs = [3, 67, 195]   # feat col offset of h1, h2, h3

    for li, (C, O) in enumerate(LAYERS, start=1):
        hin = hcols[li - 1]
        wnT, zcT, bl = gw[li - 1]
        gdt = f32 if li == 1 else bf16
        Yd = dr.tile([N, O], gdt, tag=f"Y{li}")

        # ---- xx = rowwise |h|^2 (via ones-vector matmul of h^2), store -xx/2
        # xx4 rows 0/32/64/96 (legal ACT bases) hold 1024-wide chunks
        xx4 = lb.tile([97, 1024], f32, tag="xx4")
        nc.vector.memset(xx4[:], 0.0)
        for j_ in range(8):
            sqc = wk.tile([C, 512], f32, tag="sqc")
            nc.scalar.activation(sqc[:], hin[:, j_*512:(j_+1)*512], Square)
            xp = pms.tile([1, 512], f32, tag="pms")
            nc.tensor.matmul(xp[:], _r(ones_c[0:C, :]), _r(sqc[:]),
                             start=True, stop=True)
            row = 32 * (j_ // 2)
            nc.scalar.activation(xx4[row:row+1, (j_ % 2)*512:(j_ % 2 + 1)*512],
                                 xp[:], Copy, scale=-0.5)

        # ---- Y rows -> DRAM (gather source)
        for t in range(NT):
            yp = pms.tile([128, O], f32, tag="pms")
            nc.tensor.matmul(yp[:], _r(hin[:, t*128:(t+1)*128]), _r(wnT[:]),
                             start=True, stop=True)
            ysb = wk.tile([128, O], gdt, tag="ysb")
            nc.scalar.activation(ysb[:], yp[:], Copy)
            nc.sync.dma_start(out=Yd[t*128:(t+1)*128, :], in_=ysb[:])

        # ---- distances + top-20 per query tile
        idxa = lb.tile([128, NT, K], u16, tag="idxa")
        for t in range(NT):
            st = sp.tile([128, N], f32, tag="st")
            for j_ in range(8):
                dp = pdp.tile([128, 512], f32, tag="dp")
                nc.tensor.matmul(dp[:], _r(hin[:, t*128:(t+1)*128]),
                                 _r(hin[:, j_*512:(j_+1)*512]),
                                 start=True, stop=False)
                nc.tensor.matmul(dp[:], _r(selx[:, j_ // 2, :]),
                                 _r(xx4[:, (j_ % 2)*512:(j_ % 2 + 1)*512]),
                                 start=False, stop=True)
                nc.scalar.activation(st[:, j_*512:(j_+1)*512], dp[:], Copy)
            m8 = wk.tile([128, 8], f32, tag="m8")
            idxb = wk.tile([128, 8], u16, tag="idxb")
            for r_ in range(3):
                nc.vector.max(m8[:], st[:])
                if r_ < 2:
                    nc.vector.max_index(idxa[:, t, r_*8:(r_+1)*8], m8[:], st[:])
                    nc.vector.match_replace(st[:], m8[:], st[:], imm_value=NEG)
                else:
                    nc.vector.max_index(idxb[:], m8[:], st[:])
                    nc.vector.tensor_copy(idxa[:, t, 16:20], idxb[:, 0:4])

        # ---- shuffle idx into wrapped+replicated layout for dma_gather
        idxw = lb.tile([128, NT, K, 8], i16, tag="idxw")
        for g in range(8):
            nc.sync.dma_start(out=idxw[0:16, :, :, g],
                              in_=idxa[g*16:(g+1)*16, :, :].bitcast(i16))
        nc.sync.dma_start(out=idxw[16:32, :, :, :], in_=idxw[0:16, :, :, :])
        nc.sync.dma_start(out=idxw[32:64, :, :, :], in_=idxw[0:32, :, :, :])
        nc.sync.dma_start(out=idxw[64:128, :, :, :], in_=idxw[0:64, :, :, :])

        # ---- gather Y rows of the 20 nn, max over k, add Z, leaky
        fo = foffs[li - 1]
        for t in range(NT):
            gt = gtp.tile([128, K, O], gdt, tag="gt")
            for kg in range(4):
                nc.gpsimd.dma_gather(
                    out_ap=gt[:, kg*5:(kg+1)*5, :], in_ap=Yd[:],
                    idxs_ap=idxw[:, t, kg*5:(kg+1)*5, :],
                    num_idxs=640, num_idxs_reg=640, elem_size=O,
                    queue_num=kg)
            zp = pms.tile([128, O], f32, tag="pms")
            nc.tensor.matmul(zp[:], _r(hin[:, t*128:(t+1)*128]), _r(zcT[:]),
                             start=True, stop=False)
            nc.tensor.matmul(zp[:], _r(ones_r[:]), _r(bl[:]),
                             start=False, stop=True)
            nc.vector.tensor_tensor(gt[:, 0:10, :], gt[:, 0:10, :],
                                    gt[:, 10:20, :], op=Amax)
            nc.vector.tensor_tensor(gt[:, 0:5, :], gt[:, 0:5, :],
                                    gt[:, 5:10, :], op=Amax)
            nc.vector.tensor_tensor(gt[:, 0:2, :], gt[:, 0:2, :],
                                    gt[:, 2:4, :], op=Amax)
            nc.vector.tensor_tensor(gt[:, 0:1, :], gt[:, 0:1, :],
                                    gt[:, 1:2, :], op=Amax)
            gm = wk.tile([128, O], f32, tag="gm")
            nc.vector.tensor_tensor(gm[:].unsqueeze(1), gt[:, 0:1, :],
                                    gt[:, 4:5, :], op=Amax)
            ez = wk.tile([128, O], f32, tag="ez")
            nc.vector.tensor_tensor(ez[:], gm[:], zp[:], op=Aadd)
            lt = wk.tile([128, O], f32, tag="lt")
            nc.vector.tensor_scalar(lt[:], ez[:], SLOPE, None, Amult)
            hrow = wk.tile([128, O], f32, tag="hrow")
            nc.vector.tensor_tensor(hrow[:], lt[:], ez[:], op=Amax)
            nc.scalar.activation(feat[:, t, fo:fo+O], hrow[:], Copy)
            if li < 3:
                tp = pms.tile([O, 128], f32, tag="pms")
                nc.tensor.transpose(tp[:], hrow[:], eye[:])
                nc.scalar.activation(hcols[li][:, t*128:(t+1)*128], tp[:], Copy)

    # ---- weighted pooling onto joints
    pps = pac.tile([J, 451], f32, tag="pacc")
    for t in range(NT):
        nc.tensor.matmul(pps[:], wttb[:, t, :], feat[:, t, :],
                         start=(t == 0), stop=(t == NT - 1))
    pn = sb1.tile([J, 512], f32, tag="pn")
    nc.vector.memset(pn[:, 448:512], 0.0)
    nc.scalar.activation(pn[:, 0:451], pps[:], Copy, scale=rw[:])

    # ---- skeleton edge convs (fixed ring idx via one-hot matmuls) + MLP
    def transpose_cols(src, width, tag):
        # src [J, width] -> list of [<=128, J] sbuf tiles (col chunks of 128)
        outs_ = []
        nch = (width + 127) // 128
        for c in range(nch):
            w = min(128, width - c * 128)
            tp = psk.tile([128, J], f32, tag="psk")
            nc.tensor.transpose(tp[0:w, :], src[:, c*128:c*128+w], eye[0:J, 0:J])
            ts = skt.tile([128, J], f32, tag="sktT")
            nc.scalar.activation(ts[0:w, :], tp[0:w, :], Copy)
            outs_.append((ts, w))
        return outs_

    vcat = sb1.tile([J, 448], f32, tag="vcat")

    src, swidth = pn, 512
    voffs = [(0, 256), (256, 128), (384, 64)]
    for si in range(3):
        wnT_, zcT_, b_ = sw_[si]
        co = [256, 128, 64][si]
        srcT = transpose_cols(src, swidth, f"sk{si}")
        ap_ = psk.tile([J, co], f32, tag="psk")
        for c, (ts, w) in enumerate(srcT):
            nc.tensor.matmul(ap_[:], _r(ts[0:w, :]), _r(wnT_[0:w, c, :]),
                             start=(c == 0), stop=(c == len(srcT) - 1))
        asb = skw.tile([J, 512], f32, tag="skt")
        nc.scalar.activation(asb[:, 0:co], ap_[:], Copy)
        zp_ = psk.tile([J, co], f32, tag="psk")
        for c, (ts, w) in enumerate(srcT):
            nc.tensor.matmul(zp_[:], _r(ts[0:w, :]), _r(zcT_[0:w, c, :]),
                             start=(c == 0), stop=False)
        nc.tensor.matmul(zp_[:], _r(ones_r[:, 0:J]), _r(b_[:]),
                         start=False, stop=True)
        bks = []
        for k_ in range(4):
            bp = psk.tile([J, co], f32, tag="psk")
            nc.tensor.matmul(bp[:], _r(skT[:, k_, :]), _r(asb[:, 0:co]),
                             start=True, stop=True)
            bs = skw.tile([J, 512], f32, tag="skt")
            nc.scalar.activation(bs[:, 0:co], bp[:], Copy)
            bks.append(bs)
        m01 = skw.tile([J, 512], f32, tag="skt")
        nc.vector.tensor_tensor(m01[:, 0:co], bks[0][:, 0:co], bks[1][:, 0:co], op=Amax)
        m23 = skw.tile([J, 512], f32, tag="skt")
        nc.vector.tensor_tensor(m23[:, 0:co], bks[2][:, 0:co], bks[3][:, 0:co], op=Amax)
        mm_ = skw.tile([J, 512], f32, tag="skt")
        nc.vector.tensor_tensor(mm_[:, 0:co], m01[:, 0:co], m23[:, 0:co], op=Amax)
        ez_ = skw.tile([J, 512], f32, tag="skt")
        nc.vector.tensor_tensor(ez_[:, 0:co], mm_[:, 0:co], zp_[:], op=Aadd)
        lt_ = skw.tile([J, 512], f32, tag="skt")
        vo, vw = voffs[si]
        _leaky(nc, vcat[:, vo:vo+vw], ez_[:, 0:co], lt_[:, 0:co])
        src, swidth = None, None
        if si < 2:
            vsb = skw.tile([J, 512], f32, tag="skt")
            nc.vector.tensor_copy(vsb[:, 0:co], vcat[:, vo:vo+vw])
            src, swidth = vsb[:, 0:co], co

    # ---- joint MLP: 448 -> 512 -> 256 -> 3
    def mlp_layer(src, swidth, wT, bias, co, tag, leaky=True):
        srcT = transpose_cols(src, swidth, tag)
        hp = psk.tile([J, co], f32, tag="psk")
        for c, (ts, w) in enumerate(srcT):
            nc.tensor.matmul(hp[:], _r(ts[0:w, :]), _r(wT[0:w, c, :]),
                             start=(c == 0), stop=False)
        nc.tensor.matmul(hp[:], _r(ones_r[:, 0:J]), _r(bias[:]),
                         start=False, stop=True)
        hs = skw.tile([J, 512], f32, tag="skt")
        if leaky:
            lt_ = skw.tile([J, 512], f32, tag="skt")
            nc.vector.tensor_scalar(lt_[:, 0:co], hp[:], SLOPE, None, Amult)
            nc.vector.tensor_tensor(hs[:, 0:co], lt_[:, 0:co], hp[:], op=Amax)
        else:
            nc.scalar.activation(hs[:, 0:co], hp[:], Copy)
        return hs[:, 0:co]

    h1 = mlp_layer(vcat[:], 448, m1T, m1b, 512, "m1")
    h2 = mlp_layer(h1, 512, m2T, m2b, 256, "m2")
    h3 = mlp_layer(h2, 256, m3T, m3b, 3, "m3", leaky=False)
    nc.sync.dma_start(out=outs["joints"].ap(), in_=h3)
    ctx.close()


def build_program():
    nc = bacc.Bacc("TRN2", target_bir_lowering=False, debug=False,
                   enable_asserts=False, num_devices=8, num_swdge_queues=4)
    ins = {}

    def din(name, shape, dtype=f32):
        ins[name] = nc.dram_tensor(name, shape, dtype, kind="ExternalInput")

    din("eye", [128, 128])
    din("selx", [97, 4, 128])
    din("vt", [3, N])
    din("vrows", [128, NT, 3])
    din("wtt", [128, NT, J])
    din("rw", [J, 1])
    for li, (C, O) in enumerate(LAYERS, start=1):
        din(f"g{li}_wnT", [C, O]); din(f"g{li}_zcT", [C, O]); din(f"g{li}_b", [1, O])
    din("skT", [J, 4, J])
    for si, (ci, co) in enumerate([(512, 256), (256, 128), (128, 64)], start=1):
        din(f"s{si}_wnT", [128, ci // 128, co]); din(f"s{si}_zcT", [128, ci // 128, co]); din(f"s{si}_b", [1, co])
    din("m1T", [128, 4, 512]); din("m1b", [1, 512])
    din("m2T", [128, 4, 256]); din("m2b", [1, 256])
    din("m3T", [128, 2, 3]); din("m3b", [1, 3])
    outs = {"joints": nc.dram_tensor("joints", [J, 3], f32, kind="ExternalOutput")}

    with tile.TileContext(nc) as tc:
        _emit(nc, tc, ins, outs)
    nc.compile()
    return nc


def prep_core_inputs(V, W, ring, p):
    """Host-side layout prep for one sample. p = dict of all param arrays."""
    def f(x):
        return np.ascontiguousarray(np.asarray(x, dtype=np.float32))

    def kchunks(a, kpad):
        out = np.zeros((kpad, a.shape[1]), dtype=np.float32)
        out[:a.shape[0]] = a
        return np.ascontiguousarray(
            out.reshape(kpad // 128, 128, a.shape[1]).transpose(1, 0, 2))

    d = {"eye": np.eye(128, dtype=np.float32)}
    selx = np.zeros((97, 4, 128), dtype=np.float32)
    for q in range(4):
        selx[32 * q, q, :] = 1.0
    d["selx"] = selx
    d["vt"] = f(V.T)
    d["vrows"] = f(V.reshape(NT, 128, 3).transpose(1, 0, 2))
    d["wtt"] = f(W.T.reshape(NT, 128, J).transpose(1, 0, 2))
    d["rw"] = f(1.0 / (W.sum(axis=-1, keepdims=True) + 1e-5))
    for li, (C, O) in enumerate(LAYERS, start=1):
        w = np.asarray(p[f"g{li}_w"], dtype=np.float32)
        wn, wc = w[:, :C], w[:, C:]
        d[f"g{li}_wnT"] = f(wn.T)
        d[f"g{li}_zcT"] = f((wc - wn).T)
        d[f"g{li}_b"] = f(p[f"g{li}_b"]).reshape(1, O)
    ring = np.asarray(ring).astype(np.int64)
    skT = np.zeros((J, 4, J), dtype=np.float32)
    for j in range(J):
        for k in range(4):
            skT[ring[j, k], k, j] = 1.0
    d["skT"] = skT
    sdims = [(451, 256, 512), (256, 128, 256), (128, 64, 128)]
    for si, (ci, co, cpad) in enumerate(sdims, start=1):
        w = np.asarray(p[f"s{si}_w"], dtype=np.float32)
        wn, wc = w[:, :ci], w[:, ci:]
        wnT = wn.T
        zcT = (wc - wn).T
        d[f"s{si}_wnT"] = kchunks(wnT, cpad)
        d[f"s{si}_zcT"] = kchunks(zcT, cpad)
        d[f"s{si}_b"] = f(p[f"s{si}_b"]).reshape(1, co)
    d["m1T"] = kchunks(f(np.asarray(p["m1_w"]).T), 512)
    d["m1b"] = f(p["m1_b"]).reshape(1, 512)
    d["m2T"] = kchunks(f(np.asarray(p["m2_w"]).T), 512)
    d["m2b"] = f(p["m2_b"]).reshape(1, 256)
    d["m3T"] = kchunks(f(np.asarray(p["m3_w"]).T), 256)
    d["m3b"] = f(p["m3_b"]).reshape(1, 3)
    return d


_CACHE = {}


def _get_program():
    if "nc" not in _CACHE:
        _CACHE["nc"] = build_program()
    return _CACHE["nc"]


def make_in_maps(inputs):
    p = {k: v for k, v in inputs.items()
         if k not in ("V", "W", "skeletonOneRingIdx")}
    maps = []
    for c in range(8):
        b = c % 4
        maps.append(prep_core_inputs(inputs["V"][b], inputs["W"][b],
                                     inputs["skeletonOneRingIdx"][b], p))
    return maps


def kernel(**inputs):
    nc = _get_program()
    in_maps = make_in_maps(inputs)
    res = run_bass_kernel_spmd(nc, in_maps, list(range(8)))
    out = np.stack([res.results[c]["joints"] for c in range(4)], axis=0)
    return out.astype(np.float32)


if __name__ == "__main__":
    import reference
    inp = {k: np.asarray(v) for k, v in reference.setup_inputs().items()}
    out = kernel(**inp)
    exp = np.asarray(reference.reference(**reference.setup_inputs()))
    err = np.abs(out - exp).max() / (np.abs(exp).max() + 1e-9)
    print("Relative error:", err)



# revision 17
# speedup vs baseline: 21.5751x; 21.5751x over previous
"""JointNet (DGCNN geoNet + skeleton edge convs + joint MLP) on 8 trn2 cores.

Sharding: data-parallel over batch (B=4). Core c computes sample c % 4
end-to-end; cores 4-7 are redundant replicas (outputs of cores 0-3 are used).

Key kernel ideas:
- kNN: per 128-query tile, s[i,j] = h_i . h_j - xx_j/2 (row-constant terms
  dropped; ordering-equivalent to -distance). Matmul in float32r (full PE
  rate), with the -xx/2 term folded in as a rank-1 accumulating matmul.
  Top-20 = 3 rounds of DVE max8 / max_index / match_replace.
- Edge conv: leaky(max_k(feat @ w)) with feat = [neigh-center, center] and
  w = [wn | wc] decomposes (leaky monotone, max over k order-invariant) to
  leaky(max_k Y[idx_k] + Z) with Y = h @ wn.T (gathered via SWDGE dma_gather
  from HBM) and Z = h @ (wc-wn).T + b.
- Skeleton stage gathers via host-precomputed one-hot matmuls (J=24, tiny).
"""
import sys
if "/opt/trn_rl_repo" not in sys.path:
    sys.path.insert(0, "/opt/trn_rl_repo")

import numpy as np

import concourse.bacc as bacc
import concourse.tile_utils as tile_utils
tile_utils.max_sbuf_usage = 204 * 1024
import concourse.mybir as mybir
import concourse.tile as tile
from concourse.bass_utils import run_bass_kernel_spmd

f32 = mybir.dt.float32
f32r = mybir.dt.float32r
bf16 = mybir.dt.bfloat16
fp16 = mybir.dt.float16
u16 = mybir.dt.uint16
i16 = mybir.dt.int16

N = 4096
NT = 32            # query tiles of 128
K = 20
J = 24
SLOPE = 0.2
LAYERS = [(3, 64), (64, 128), (128, 256)]   # (C_in, O) for g1..g3
NEG = -3.0e38
NEG16 = -60000.0

Copy = mybir.ActivationFunctionType.Copy
Square = mybir.ActivationFunctionType.Square
Amax = mybir.AluOpType.max
Aadd = mybir.AluOpType.add
Amult = mybir.AluOpType.mult


def _r(ap):
    return ap  # plain fp32 matmul (f32r/bitcast breaks NEFF lowering)


def _leaky(nc, out_ap, in_ap, tmp_ap):
    # out = max(0.2*x, x)
    nc.vector.tensor_scalar(tmp_ap, in_ap, SLOPE, None, Amult)
    nc.vector.tensor_tensor(out_ap, tmp_ap, in_ap, op=Amax)


def _emit(nc, tc, ins, outs):
    from contextlib import ExitStack
    ctx = ExitStack()
    sb1 = ctx.enter_context(tc.tile_pool(name="pers", bufs=1))
    sp = ctx.enter_context(tc.tile_pool(name="sp", bufs=2))
    wk = ctx.enter_context(tc.tile_pool(name="wk", bufs=2))
    gtp = ctx.enter_context(tc.tile_pool(name="gtp", bufs=2))
    skw = ctx.enter_context(tc.tile_pool(name="skw", bufs=6))
    skt = ctx.enter_context(tc.tile_pool(name="skt", bufs=4))
    lb = ctx.enter_context(tc.tile_pool(name="lb", bufs=1))
    pdp = ctx.enter_context(tc.tile_pool(name="pdp", bufs=3, space="PSUM"))
    pms = ctx.enter_context(tc.tile_pool(name="pms", bufs=2, space="PSUM"))
    pac = ctx.enter_context(tc.tile_pool(name="pac", bufs=1, space="PSUM"))
    psk = ctx.enter_context(tc.tile_pool(name="psk", bufs=2, space="PSUM"))
    dr = ctx.enter_context(tc.tile_pool(name="dr", bufs=1, space="DRAM"))

    def load(name, shape, tag=None):
        t = sb1.tile(shape, f32, tag=tag or name)
        nc.sync.dma_start(out=t[:], in_=ins[name].ap())
        return t

    eye = load("eye", [128, 128])
    vt = load("vt", [3, N])
    vrows = load("vrows", [128, NT, 3])
    wtt = load("wtt", [128, NT, J])
    rw = load("rw", [J, 1])
    gw = []
    for li, (C, O) in enumerate(LAYERS, start=1):
        gw.append((load(f"g{li}_wnT", [C, O]), load(f"g{li}_zcT", [C, O]),
                   load(f"g{li}_b", [1, O])))
    skT = load("skT", [J, 4, J])
    sw_ = []
    for si, (ci, co) in enumerate([(512, 256), (256, 128), (128, 64)], start=1):
        sw_.append((load(f"s{si}_wnT", [128, ci // 128, co]),
                    load(f"s{si}_zcT", [128, ci // 128, co]),
                    load(f"s{si}_b", [1, co])))
    m1T = load("m1T", [128, 4, 512]); m1b = load("m1b", [1, 512])
    m2T = load("m2T", [128, 4, 256]); m2b = load("m2b", [1, 256])
    m3T = load("m3T", [128, 2, 3]); m3b = load("m3b", [1, 3])

    ones_r = sb1.tile([1, 128], f32, tag="ones_r")
    nc.vector.memset(ones_r[:], 1.0)
    ones_rb = sb1.tile([1, 128], bf16, tag="ones_rb")
    nc.vector.memset(ones_rb[:], 1.0)
    ones_c = sb1.tile([128, 1], f32, tag="ones_c")
    nc.vector.memset(ones_c[:], 1.0)
    wttb = sb1.tile([128, NT, J], bf16, tag="wttb")
    nc.vector.tensor_copy(wttb[:], wtt[:])

    feat = sb1.tile([128, NT, 451], bf16, tag="feat")
    nc.vector.tensor_copy(feat[:, :, 0:3], vrows[:])

    # bf16 copies of geo weights (PE runs bf16 at 4x the fp32 row rate)
    gwb = []
    for li, (C, O) in enumerate(LAYERS, start=1):
        wnT, zcT, bl = gw[li - 1]
        wnTb = sb1.tile([C, O], bf16, tag=f"wnTb{li}")
        nc.vector.tensor_copy(wnTb[:], wnT[:])
        zcTb = sb1.tile([C, O], bf16, tag=f"zcTb{li}")
        nc.vector.tensor_copy(zcTb[:], zcT[:])
        blb = sb1.tile([1, O], bf16, tag=f"blb{li}")
        nc.vector.tensor_copy(blb[:], bl[:])
        gwb.append((wnTb, zcTb, blb))

    vb = sb1.tile([3, N], bf16, tag="vb")
    nc.vector.tensor_copy(vb[:], vt[:])
    hcol1 = sb1.tile([64, N], bf16, tag="hcol1")
    hcol2 = sb1.tile([128, N], bf16, tag="hcol2")
    hcols = [vb, hcol1, hcol2, None]
    foffs = [3, 67, 195]   # feat col offset of h1, h2, h3

    for li, (C, O) in enumerate(LAYERS, start=1):
        hin = hcols[li - 1]
        wnTb, zcTb, blb = gwb[li - 1]
        gdt = f32 if li == 1 else bf16
        Yd = dr.tile([N, O], gdt, tag=f"Y{li}")

        # ---- xxr[0, j] = -|h_j|^2/2 (ones-vector matmul of h^2)
        xxr = lb.tile([1, N], bf16, tag="xxr")
        for j_ in range(8):
            sqc = wk.tile([C, 512], f32, tag="sqc")
            nc.scalar.activation(sqc[:], hin[:, j_*512:(j_+1)*512], Square)
            xp = pms.tile([1, 512], f32, tag="pms")
            nc.tensor.matmul(xp[:], ones_c[0:C, :], sqc[:],
                             start=True, stop=True)
            nc.scalar.activation(xxr[:, j_*512:(j_+1)*512], xp[:], Copy,
                                 scale=-0.5)

        # ---- Y rows -> DRAM (gather source)
        for t in range(NT):
            yp = pms.tile([128, O], f32, tag="pms")
            nc.tensor.matmul(yp[:], hin[:, t*128:(t+1)*128], wnTb[:],
                             start=True, stop=True)
            ysb = wk.tile([128, O], gdt, tag="ysb")
            nc.scalar.activation(ysb[:], yp[:], Copy)
            nc.sync.dma_start(out=Yd[t*128:(t+1)*128, :], in_=ysb[:])

        # ---- distances (fp16) + hierarchical top-20 per query tile
        # st = h_i.h_j - xx_j/2 in fp16; 8 member planes of 512 columns.
        # Group g = {g + 512*m}: group-max tree (fp16 TT, 2x mode), then
        # max8/match_replace rounds on the 512-wide group-max, and 3
        # value-keyed max_index passes on full st recover the columns.
        idxa = lb.tile([128, NT, K], u16, tag="idxa")
        for t in range(NT):
            st = sp.tile([128, N], fp16, tag="st")
            for j_ in range(8):
                dp = pdp.tile([128, 512], f32, tag="dp")
                nc.tensor.matmul(dp[:], hin[:, t*128:(t+1)*128],
                                 hin[:, j_*512:(j_+1)*512],
                                 start=True, stop=False)
                nc.tensor.matmul(dp[:], ones_rb[:],
                                 xxr[:, j_*512:(j_+1)*512],
                                 start=False, stop=True)
                nc.scalar.activation(st[:, j_*512:(j_+1)*512], dp[:], Copy)
            t1 = wk.tile([128, 2048], fp16, tag="t1")
            nc.vector.tensor_tensor(t1[:], st[:, 0:2048], st[:, 2048:4096],
                                    op=Amax)
            t2 = wk.tile([128, 1024], fp16, tag="t2")
            nc.vector.tensor_tensor(t2[:], t1[:, 0:1024], t1[:, 1024:2048],
                                    op=Amax)
            gm = wk.tile([128, 512], fp16, tag="gm")
            nc.vector.tensor_tensor(gm[:], t2[:, 0:512], t2[:, 512:1024],
                                    op=Amax)
            m8 = wk.tile([128, 8], fp16, tag="m8")
            idxb = wk.tile([128, 8], u16, tag="idxb")
            for r_ in range(3):
                nc.vector.max(m8[:], gm[:])
                if r_ < 2:
                    nc.vector.max_index(idxa[:, t, r_*8:(r_+1)*8], m8[:], st[:])
                    nc.vector.match_replace(gm[:], m8[:], gm[:], imm_value=NEG16)
                else:
                    nc.vector.max_index(idxb[:], m8[:], st[:])
                    nc.vector.tensor_copy(idxa[:, t, 16:20], idxb[:, 0:4])

        # ---- shuffle idx into wrapped+replicated layout for dma_gather
        idxw = lb.tile([128, NT, K, 8], i16, tag="idxw")
        for g in range(8):
            nc.sync.dma_start(out=idxw[0:16, :, :, g],
                              in_=idxa[g*16:(g+1)*16, :, :].bitcast(i16))
        nc.sync.dma_start(out=idxw[16:32, :, :, :], in_=idxw[0:16, :, :, :])
        nc.sync.dma_start(out=idxw[32:64, :, :, :], in_=idxw[0:32, :, :, :])
        nc.sync.dma_start(out=idxw[64:128, :, :, :], in_=idxw[0:64, :, :, :])

        # ---- gather Y rows of the 20 nn, max over k, add Z, leaky
        fo = foffs[li - 1]
        for t in range(NT):
            gt = gtp.tile([128, K, O], gdt, tag="gt")
            for kg in range(4):
                nc.gpsimd.dma_gather(
                    out_ap=gt[:, kg*5:(kg+1)*5, :], in_ap=Yd[:],
                    idxs_ap=idxw[:, t, kg*5:(kg+1)*5, :],
                    num_idxs=640, num_idxs_reg=640, elem_size=O,
                    queue_num=kg)
            zp = pms.tile([128, O], f32, tag="pms")
            nc.tensor.matmul(zp[:], hin[:, t*128:(t+1)*128], zcTb[:],
                             start=True, stop=False)
            nc.tensor.matmul(zp[:], ones_rb[:], blb[:],
                             start=False, stop=True)
            nc.vector.tensor_tensor(gt[:, 0:10, :], gt[:, 0:10, :],
                                    gt[:, 10:20, :], op=Amax)
            nc.vector.tensor_tensor(gt[:, 0:5, :], gt[:, 0:5, :],
                                    gt[:, 5:10, :], op=Amax)
            nc.vector.tensor_tensor(gt[:, 0:2, :], gt[:, 0:2, :],
                                    gt[:, 2:4, :], op=Amax)
            nc.vector.tensor_tensor(gt[:, 0:1, :], gt[:, 0:1, :],
                                    gt[:, 1:2, :], op=Amax)
            gmx = wk.tile([128, O], f32, tag="gmx")
            nc.vector.tensor_tensor(gmx[:].unsqueeze(1), gt[:, 0:1, :],
                                    gt[:, 4:5, :], op=Amax)
            ez = wk.tile([128, O], f32, tag="ez")
            nc.vector.tensor_tensor(ez[:], gmx[:], zp[:], op=Aadd)
            hrow = wk.tile([128, O], f32, tag="hrow")
            nc.vector.scalar_tensor_tensor(hrow[:], ez[:], SLOPE, ez[:],
                                           op0=Amult, op1=Amax)
            nc.scalar.activation(feat[:, t, fo:fo+O], hrow[:], Copy)
            if li < 3:
                tp = pms.tile([O, 128], f32, tag="pms")
                nc.tensor.transpose(tp[:], hrow[:], eye[:])
                nc.scalar.activation(hcols[li][:, t*128:(t+1)*128], tp[:], Copy)

    # ---- weighted pooling onto joints
    pps = pac.tile([J, 451], f32, tag="pacc")
    for t in range(NT):
        nc.tensor.matmul(pps[:], wttb[:, t, :], feat[:, t, :],
                         start=(t == 0), stop=(t == NT - 1))
    pn = sb1.tile([J, 512], f32, tag="pn")
    nc.vector.memset(pn[:, 448:512], 0.0)
    nc.scalar.activation(pn[:, 0:451], pps[:], Copy, scale=rw[:])

    # ---- skeleton edge convs (fixed ring idx via one-hot matmuls) + MLP
    def transpose_cols(src, width, tag):
        # src [J, width] -> list of [<=128, J] sbuf tiles (col chunks of 128)
        outs_ = []
        nch = (width + 127) // 128
        for c in range(nch):
            w = min(128, width - c * 128)
            tp = psk.tile([128, J], f32, tag="psk")
            nc.tensor.transpose(tp[0:w, :], src[:, c*128:c*128+w], eye[0:J, 0:J])
            ts = skt.tile([128, J], f32, tag="sktT")
            nc.scalar.activation(ts[0:w, :], tp[0:w, :], Copy)
            outs_.append((ts, w))
        return outs_

    vcat = sb1.tile([J, 448], f32, tag="vcat")

    src, swidth = pn, 512
    voffs = [(0, 256), (256, 128), (384, 64)]
    for si in range(3):
        wnT_, zcT_, b_ = sw_[si]
        co = [256, 128, 64][si]
        srcT = transpose_cols(src, swidth, f"sk{si}")
        ap_ = psk.tile([J, co], f32, tag="psk")
        for c, (ts, w) in enumerate(srcT):
            nc.tensor.matmul(ap_[:], _r(ts[0:w, :]), _r(wnT_[0:w, c, :]),
                             start=(c == 0), stop=(c == len(srcT) - 1))
        asb = skw.tile([J, 512], f32, tag="skt")
        nc.scalar.activation(asb[:, 0:co], ap_[:], Copy)
        zp_ = psk.tile([J, co], f32, tag="psk")
        for c, (ts, w) in enumerate(srcT):
            nc.tensor.matmul(zp_[:], _r(ts[0:w, :]), _r(zcT_[0:w, c, :]),
                             start=(c == 0), stop=False)
        nc.tensor.matmul(zp_[:], _r(ones_r[:, 0:J]), _r(b_[:]),
                         start=False, stop=True)
        bks = []
        for k_ in range(4):
            bp = psk.tile([J, co], f32, tag="psk")
            nc.tensor.matmul(bp[:], _r(skT[:, k_, :]), _r(asb[:, 0:co]),
                             start=True, stop=True)
            bs = skw.tile([J, 512], f32, tag="skt")
            nc.scalar.activation(bs[:, 0:co], bp[:], Copy)
            bks.append(bs)
        m01 = skw.tile([J, 512], f32, tag="skt")
        nc.vector.tensor_tensor(m01[:, 0:co], bks[0][:, 0:co], bks[1][:, 0:co], op=Amax)
        m23 = skw.tile([J, 512], f32, tag="skt")
        nc.vector.tensor_tensor(m23[:, 0:co], bks[2][:, 0:co], bks[3][:, 0:co], op=Amax)
        mm_ = skw.tile([J, 512], f32, tag="skt")
        nc.vector.tensor_tensor(mm_[:, 0:co], m01[:, 0:co], m23[:, 0:co], op=Amax)
        ez_ = skw.tile([J, 512], f32, tag="skt")
        nc.vector.tensor_tensor(ez_[:, 0:co], mm_[:, 0:co], zp_[:], op=Aadd)
        lt_ = skw.tile([J, 512], f32, tag="skt")
        vo, vw = voffs[si]
        _leaky(nc, vcat[:, vo:vo+vw], ez_[:, 0:co], lt_[:, 0:co])
        src, swidth = None, None
        if si < 2:
            vsb = skw.tile([J, 512], f32, tag="skt")
            nc.vector.tensor_copy(vsb[:, 0:co], vcat[:, vo:vo+vw])
            src, swidth = vsb[:, 0:co], co

    # ---- joint MLP: 448 -> 512 -> 256 -> 3
    def mlp_layer(src, swidth, wT, bias, co, tag, leaky=True):
        srcT = transpose_cols(src, swidth, tag)
        hp = psk.tile([J, co], f32, tag="psk")
        for c, (ts, w) in enumerate(srcT):
            nc.tensor.matmul(hp[:], _r(ts[0:w, :]), _r(wT[0:w, c, :]),
                             start=(c == 0), stop=False)
        nc.tensor.matmul(hp[:], _r(ones_r[:, 0:J]), _r(bias[:]),
                         start=False, stop=True)
        hs = skw.tile([J, 512], f32, tag="skt")
        if leaky:
            lt_ = skw.tile([J, 512], f32, tag="skt")
            nc.vector.tensor_scalar(lt_[:, 0:co], hp[:], SLOPE, None, Amult)
            nc.vector.tensor_tensor(hs[:, 0:co], lt_[:, 0:co], hp[:], op=Amax)
        else:
            nc.scalar.activation(hs[:, 0:co], hp[:], Copy)
        return hs[:, 0:co]

    h1 = mlp_layer(vcat[:], 448, m1T, m1b, 512, "m1")
    h2 = mlp_layer(h1, 512, m2T, m2b, 256, "m2")
    h3 = mlp_layer(h2, 256, m3T, m3b, 3, "m3", leaky=False)
    nc.sync.dma_start(out=outs["joints"].ap(), in_=h3)
    ctx.close()


def build_program():
    nc = bacc.Bacc("TRN2", target_bir_lowering=False, debug=False,
                   enable_asserts=False, num_devices=8, num_swdge_queues=4)
    ins = {}

    def din(name, shape, dtype=f32):
        ins[name] = nc.dram_tensor(name, shape, dtype, kind="ExternalInput")

    din("eye", [128, 128])
    din("vt", [3, N])
    din("vrows", [128, NT, 3])
    din("wtt", [128, NT, J])
    din("rw", [J, 1])
    for li, (C, O) in enumerate(LAYERS, start=1):
        din(f"g{li}_wnT", [C, O]); din(f"g{li}_zcT", [C, O]); din(f"g{li}_b", [1, O])
    din("skT", [J, 4, J])
    for si, (ci, co) in enumerate([(512, 256), (256, 128), (128, 64)], start=1):
        din(f"s{si}_wnT", [128, ci // 128, co]); din(f"s{si}_zcT", [128, ci // 128, co]); din(f"s{si}_b", [1, co])
    din("m1T", [128, 4, 512]); din("m1b", [1, 512])
    din("m2T", [128, 4, 256]); din("m2b", [1, 256])
    din("m3T", [128, 2, 3]); din("m3b", [1, 3])
    outs = {"joints": nc.dram_tensor("joints", [J, 3], f32, kind="ExternalOutput")}

    with tile.TileContext(nc) as tc:
        _emit(nc, tc, ins, outs)
    nc.compile()
    return nc


def prep_core_inputs(V, W, ring, p):
    """Host-side layout prep for one sample. p = dict of all param arrays."""
    def f(x):
        return np.ascontiguousarray(np.asarray(x, dtype=np.float32))

    def kchunks(a, kpad):
        out = np.zeros((kpad, a.shape[1]), dtype=np.float32)
        out[:a.shape[0]] = a
        return np.ascontiguousarray(
            out.reshape(kpad // 128, 128, a.shape[1]).transpose(1, 0, 2))

    d = {"eye": np.eye(128, dtype=np.float32)}
    d["vt"] = f(V.T)
    d["vrows"] = f(V.reshape(NT, 128, 3).transpose(1, 0, 2))
    d["wtt"] = f(W.T.reshape(NT, 128, J).transpose(1, 0, 2))
    d["rw"] = f(1.0 / (W.sum(axis=-1, keepdims=True) + 1e-5))
    for li, (C, O) in enumerate(LAYERS, start=1):
        w = np.asarray(p[f"g{li}_w"], dtype=np.float32)
        wn, wc = w[:, :C], w[:, C:]
        d[f"g{li}_wnT"] = f(wn.T)
        d[f"g{li}_zcT"] = f((wc - wn).T)
        d[f"g{li}_b"] = f(p[f"g{li}_b"]).reshape(1, O)
    ring = np.asarray(ring).astype(np.int64)
    skT = np.zeros((J, 4, J), dtype=np.float32)
    for j in range(J):
        for k in range(4):
            skT[ring[j, k], k, j] = 1.0
    d["skT"] = skT
    sdims = [(451, 256, 512), (256, 128, 256), (128, 64, 128)]
    for si, (ci, co, cpad) in enumerate(sdims, start=1):
        w = np.asarray(p[f"s{si}_w"], dtype=np.float32)
        wn, wc = w[:, :ci], w[:, ci:]
        wnT = wn.T
        zcT = (wc - wn).T
        d[f"s{si}_wnT"] = kchunks(wnT, cpad)
        d[f"s{si}_zcT"] = kchunks(zcT, cpad)
        d[f"s{si}_b"] = f(p[f"s{si}_b"]).reshape(1, co)
    d["m1T"] = kchunks(f(np.asarray(p["m1_w"]).T), 512)
    d["m1b"] = f(p["m1_b"]).reshape(1, 512)
    d["m2T"] = kchunks(f(np.asarray(p["m2_w"]).T), 512)
    d["m2b"] = f(p["m2_b"]).reshape(1, 256)
    d["m3T"] = kchunks(f(np.asarray(p["m3_w"]).T), 256)
    d["m3b"] = f(p["m3_b"]).reshape(1, 3)
    return d


_CACHE = {}


def _get_program():
    if "nc" not in _CACHE:
        _CACHE["nc"] = build_program()
    return _CACHE["nc"]


def make_in_maps(inputs):
    p = {k: v for k, v in inputs.items()
         if k not in ("V", "W", "skeletonOneRingIdx")}
    maps = []
    for c in range(8):
        b = c % 4
        maps.append(prep_core_inputs(inputs["V"][b], inputs["W"][b],
                                     inputs["skeletonOneRingIdx"][b], p))
    return maps


def kernel(**inputs):
    nc = _get_program()
    in_maps = make_in_maps(inputs)
    res = run_bass_kernel_spmd(nc, in_maps, list(range(8)))
    out = np.stack([res.results[c]["joints"] for c in range(4)], axis=0)
    return out.astype(np.float32)


if __name__ == "__main__":
    import reference
    inp = {k: np.asarray(v) for k, v in reference.setup_inputs().items()}
    out = kernel(**inp)
    exp = np.asarray(reference.reference(**reference.setup_inputs()))
    err = np.abs(out - exp).max() / (np.abs(exp).max() + 1e-9)
    print("Relative error:", err)

